# revision 14
# baseline (speedup 1.0000x reference)
"""Multi-head attention (B=4, S=2048, D=1024, H=16, HD=64) on 8 TRN2 NeuronCores.

Sharding: core c handles batch b = c//2 and head-group hg = c%2 (8 heads each).
Attention is embarrassingly parallel over (b, head-group); the QKV projection is
column-sharded per core (tensor parallel on heads).

Per-core dataflow (everything in "transposed" layout to avoid on-chip transposes):
  - Host passes X^T [D, S] (f32), W slices in natural [D, cols] layout.
  - Projection:  Q^T/K^T [1024, S] = W_qk^T @ X accumulated in SBUF tiles (sbt),
                 which the attention stage reads directly as Q^T/K^T;
                 V [S, 512] = X @ W_v, kept in SBUF augmented with a ones-column
                 per head (V').
  - Per head:    S^T[k,q] = K^T.T @ Q^T  (PSUM, fp32)
                 st = exp(S^T / 8)       (ScalarE, fused scale; mask is all-ones and
                                          softmax is shift-invariant => no max pass)
                 out^T[d,q], sums[q] = V'^T @ st  (ones-row of V' yields softmax sums)
                 out^T[d,q] /= sums[q]   (DVE reciprocal + gpsimd partition broadcast)
  - Host transposes per-core out^T [512, S] back and concatenates.

The projection is sliced into ~1us "quanta" (2 contraction chunks of one m-tile
half, or one V column-group of one sequence chunk) that are woven into the
attention chunk loop on a static schedule, so the in-order PE stream fills the
bubbles left by the ScalarE-paced softmax instead of running the projection as a
serial prefix. The schedule respects data deps: m-tile pair p is produced during
head 2p-1; V column-group g is produced just-in-time inside head 2g.

All matmuls run in float32r (fp32 data, ~1e-3 matmul rel err, bf16-class speed).
Projection partial sums accumulate in SBUF via DVE (float32r rounding per step).
b_qkv is applied (it is zeros in practice); mask is all-True per the problem spec
and is ignored.
"""

import numpy as np

import concourse.bass as bass
import concourse.mybir as mybir
import concourse.tile as tile
from concourse import bacc
from concourse.bass_utils import run_bass_kernel_spmd

F32 = mybir.dt.float32
F32R = mybir.dt.float32r
AF = mybir.ActivationFunctionType
ALU = mybir.AluOpType

P = 128          # partitions
D = 1024         # model dim
S = 2048         # sequence
HD = 64          # head dim
NHC = 8          # heads per core
QKC = NHC * HD   # 512 columns per core for each of Q, K, V
KD = D // P      # 8 contraction chunks
MS = S // P      # 16 sequence chunks
NQ = S // 512    # 4 q-tiles of 512
SCALE = 1.0 / 8.0  # 1/sqrt(HD)

# V is produced as a single full-width (N=512) prefix after m-tiles 0/4;
# float32r matmuls need free dim >= 256, so no narrow column groups.
VGRPS = [(0, 8)]

N_CORES = 8
B_FULL, H_FULL = 4, 16


def _build(iters=1):
    nc = bacc.Bacc(None, target_bir_lowering=False)

    xt = nc.dram_tensor("xt", [D, S], F32R, kind="ExternalInput")
    wqk = nc.dram_tensor("wqk", [D, 2 * QKC], F32R, kind="ExternalInput")
    wv = nc.dram_tensor("wv", [D, QKC], F32R, kind="ExternalInput")
    bqk = nc.dram_tensor("bqk", [2 * QKC], F32, kind="ExternalInput")
    bv = nc.dram_tensor("bv", [QKC], F32, kind="ExternalInput")
    outT = nc.dram_tensor("outT", [QKC, S], F32, kind="ExternalOutput")

    with tile.TileContext(nc) as tc:
        with (
            tc.tile_pool(name="persist", bufs=1) as pp,
            tc.tile_pool(name="sbtp", bufs=4) as sbtp,
            tc.tile_pool(name="stp", bufs=5) as stp,
            tc.tile_pool(name="psc", bufs=2, space="PSUM") as psc,
            tc.tile_pool(name="psav", bufs=4, space="PSUM") as psav,
        ):
            # bias staging: bqk_sb[p, m] = bqk[m*128 + p]; bv broadcast across partitions
            bqk_sb = pp.tile([P, KD], F32, tag="bqk", name="bqk_sb")
            nc.sync.dma_start(out=bqk_sb[:], in_=bqk[:].rearrange("(m p) -> p m", p=P))
            bv_row = pp.tile([1, QKC], F32, tag="bvr", name="bv_row")
            nc.sync.dma_start(out=bv_row[:], in_=bv[:].rearrange("(o n) -> o n", o=1))
            bv_bc = pp.tile([P, QKC], F32, tag="bvb", name="bv_bc")
            nc.gpsimd.partition_broadcast(bv_bc[:], bv_row[:])
            ones8 = pp.tile([P, NHC], F32, tag="ones8", name="ones8")
            nc.vector.memset(ones8[:], 1.0)

            for it in range(iters):
                # V' tiles: [128 seq, 8 heads, 64+1] with ones in the last column
                v_sb = [
                    pp.tile([P, NHC, HD + 1], F32R, tag=f"v{k}", name=f"v{it}_{k}")
                    for k in range(MS)
                ]

                with tc.tile_pool(name=f"proj{it}", bufs=1) as pj:
                    w_tiles = {}

                    def load_wm(m, it=it):
                        w_tiles[m] = pj.tile([P, KD, P], F32R, tag="wm", bufs=2,
                                             name=f"wm{it}_{m}")
                        nc.sync.dma_start(
                            out=w_tiles[m][:],
                            in_=wqk[:, :].rearrange("(k p) n -> p k n", p=P)[:, :, m * P:(m + 1) * P])

                    load_wm(0)
                    load_wm(4)
                    xt_sb = [pj.tile([P, S], F32R, tag=f"xt{k}", name=f"xt{it}_{k}")
                             for k in range(KD)]
                    wv_sb = [pj.tile([P, QKC], F32R, tag=f"wv{k}", name=f"wv{it}_{k}")
                             for k in range(KD)]
                    for k in range(KD):
                        nc.sync.dma_start(out=xt_sb[k][:], in_=xt[k * P:(k + 1) * P, :])
                        nc.sync.dma_start(out=wv_sb[k][:], in_=wv[k * P:(k + 1) * P, :])

                    sbt_tiles = {}

                    def qk_quantum(m, nh, kk, it=it):
                        """Accumulate k-chunks 4kk..4kk+3 of m-tile m, n-half nh
                        into the sbt SBUF accumulator (PE -> PSUM -> DVE add)."""
                        if m not in w_tiles:
                            load_wm(m)
                        if m not in sbt_tiles:
                            sbt_tiles[m] = sbtp.tile([P, S], F32R, tag="sbt",
                                                     name=f"sbt{it}_{m}")
                        w_m, sbt = w_tiles[m], sbt_tiles[m]
                        ps = psc.tile([P, 1024], F32, tag="sc", name=f"pq{it}_{m}_{nh}_{kk}")
                        ks = range(4 * kk, 4 * kk + 4)
                        for j, k in enumerate(ks):
                            nc.tensor.matmul(
                                ps[:, 0:512], w_m[:, k, :],
                                xt_sb[k][:, nh * 1024: nh * 1024 + 512],
                                start=(j == 0), stop=(j == 3))
                            nc.tensor.matmul(
                                ps[:, 512:1024], w_m[:, k, :],
                                xt_sb[k][:, nh * 1024 + 512:(nh + 1) * 1024],
                                start=(j == 0), stop=(j == 3))
                        dst = sbt[:, nh * 1024:(nh + 1) * 1024]
                        if kk == 0:
                            nc.vector.tensor_scalar_add(dst, ps[:], bqk_sb[:, m:m + 1])
                        else:
                            nc.vector.tensor_tensor(out=dst, in0=ps[:], in1=dst, op=ALU.add)

                    def v_quantum(ms, g, it=it, v_sb=v_sb):
                        """Produce V' columns for head-group g at sequence chunk ms."""
                        h0g, nh_g = VGRPS[g]
                        w = nh_g * HD
                        ps = psc.tile([P, w], F32, tag="sc", name=f"pv{it}_{ms}_{g}")
                        for k in range(KD):
                            nc.tensor.matmul(
                                ps[:], xt_sb[k][:, ms * P:(ms + 1) * P],
                                wv_sb[k][:, h0g * HD: h0g * HD + w],
                                start=(k == 0), stop=(k == KD - 1))
                        nc.vector.tensor_tensor(
                            out=v_sb[ms][:, h0g:h0g + nh_g, 0:HD],
                            in0=ps[:].rearrange("p (h e) -> p h e", e=HD),
                            in1=bv_bc[:, h0g * HD: h0g * HD + w].rearrange(
                                "p (h e) -> p h e", e=HD),
                            op=ALU.add)
                        nc.vector.tensor_copy(
                            v_sb[ms][:, h0g:h0g + nh_g, HD:HD + 1],
                            ones8[:, h0g:h0g + nh_g].rearrange("p (h o) -> p h o", o=1))

                    # ---- static quantum schedule ----
                    # sched[(h, kc)] -> quanta emitted just before that chunk's QK
                    sched = {}

                    def add(h, kc, fn):
                        sched.setdefault((h, kc), []).append(fn)

                    # V group g: chunks 0,1 pre-produced at the end of head 2g-1
                    # (upfront for g=0); the rest just-in-time inside head 2g.
                    # group 0 feeds head 0 just-in-time; group 1 (heads 4-7)
                    # is produced during head 2, well before head 4 needs it
                    for g in range(len(VGRPS)):
                        for ms in range(MS):
                            fn = (lambda ms=ms, g=g: v_quantum(ms, g))
                            if g == 0 and ms >= 2:
                                add(0, ms - 2, fn)
                            elif g == 1:
                                add(2, ms, fn)
                    # m-tile pair p (Q tile p, K tile 4+p) produced during head 2p-1
                    for p in (1, 2, 3):
                        quanta = []
                        for kk in range(2):
                            for m in (p, 4 + p):
                                for nh in range(2):
                                    quanta.append(lambda m=m, nh=nh, kk=kk: qk_quantum(m, nh, kk))
                        for i, fn in enumerate(quanta):
                            add(2 * p - 1, 2 * i, fn)

                    # upfront: m-tiles 0 and 4 (heads 0/1), V group 0 chunks 0,1
                    for kk in range(2):
                        for m in (0, 4):
                            for nh in range(2):
                                qk_quantum(m, nh, kk)
                    v_quantum(0, 0)
                    v_quantum(1, 0)

                    # ---------------- attention ----------------
                    ot_cell = [None]

                    def attention_head(h, it=it, v_sb=v_sb):
                        g = h // 2
                        off = (h % 2) * HD
                        qt = sbt_tiles[g]
                        kt = sbt_tiles[4 + g]

                        avs = [
                            psav.tile([HD + 1, 512], F32, tag="av", name=f"av{it}_{h}_{q}")
                            for q in range(NQ)
                        ]
                        def emit_av(kc, st):
                            for q in range(NQ):
                                nc.tensor.matmul(
                                    avs[q][:], v_sb[kc][:, h, :], st[:, q * 512:(q + 1) * 512],
                                    start=(kc == 0), stop=(kc == MS - 1))

                        # software pipeline: chunk kc emits QK/exp for kc but the
                        # AV matmuls for kc-1, so the in-order PE stream never
                        # waits on ScalarE finishing the current chunk's exp.
                        prev = None
                        for kc in range(MS):
                            st = stp.tile([P, S], F32R, tag="st", name=f"st{it}_{h}_{kc}")
                            for qh in range(2):
                                sc = psc.tile([P, 1024], F32, tag="sc",
                                              name=f"sc{it}_{h}_{kc}_{qh}")
                                nc.tensor.matmul(
                                    sc[:, 0:512],
                                    kt[off:off + HD, kc * P:(kc + 1) * P],
                                    qt[off:off + HD, qh * 1024: qh * 1024 + 512],
                                    start=True, stop=True)
                                nc.tensor.matmul(
                                    sc[:, 512:1024],
                                    kt[off:off + HD, kc * P:(kc + 1) * P],
                                    qt[off:off + HD, qh * 1024 + 512:(qh + 1) * 1024],
                                    start=True, stop=True)
                                nc.scalar.activation(
                                    st[:, qh * 1024:(qh + 1) * 1024], sc[:],
                                    AF.Exp, scale=SCALE)
                            if prev is not None:
                                emit_av(*prev)
                            for fn in sched.pop((h, kc), ()):
                                fn()
                            prev = (kc, st)
                        emit_av(*prev)

                        # normalize: rows 0..63 of each av tile divided by the sums row
                        bc = stp.tile([HD, S], F32, tag="st", name=f"bc{it}_{h}")
                        for q in range(NQ):
                            rec = stp.tile([1, 512], F32, tag="rec", bufs=2,
                                           name=f"rec{it}_{h}_{q}")
                            nc.vector.reciprocal(rec[:], avs[q][HD:HD + 1, :])
                            nc.gpsimd.partition_broadcast(
                                bc[:, q * 512:(q + 1) * 512], rec[:])
                        if h % 2 == 0:
                            ot_cell[0] = stp.tile([P, S], F32, tag="st",
                                                  name=f"ot{it}_{h // 2}")
                        ot_g = ot_cell[0]
                        for q in range(NQ):
                            nc.vector.tensor_mul(
                                ot_g[off:off + HD, q * 512:(q + 1) * 512],
                                avs[q][0:HD, :], bc[:, q * 512:(q + 1) * 512])
                        if h % 2 == 1:
                            gg = h // 2
                            nc.sync.dma_start(out=outT[gg * P:(gg + 1) * P, :], in_=ot_g[:])

                    for h in range(NHC):
                        attention_head(h)
                    assert not sched, f"unemitted quanta: {list(sched)}"

    nc.finalize()
    return nc


_NC_CACHE = {}


def _get_nc(iters=1):
    if iters not in _NC_CACHE:
        _NC_CACHE[iters] = _build(iters)
    return _NC_CACHE[iters]


def make_in_maps(inputs, W_qkv, b_qkv):
    inputs = np.asarray(inputs, dtype=np.float32)
    W = np.asarray(W_qkv, dtype=np.float32)
    b = np.asarray(b_qkv, dtype=np.float32)
    xt_by_b = [np.ascontiguousarray(inputs[bi].T) for bi in range(B_FULL)]
    in_maps = []
    for c in range(N_CORES):
        bi, hg = c // 2, c % 2
        c0 = hg * QKC
        in_maps.append({
            "xt": xt_by_b[bi],
            "wqk": np.ascontiguousarray(
                np.concatenate([W[:, c0:c0 + QKC], W[:, D + c0: D + c0 + QKC]], axis=1)),
            "wv": np.ascontiguousarray(W[:, 2 * D + c0: 2 * D + c0 + QKC]),
            "bqk": np.ascontiguousarray(
                np.concatenate([b[c0:c0 + QKC], b[D + c0: D + c0 + QKC]])),
            "bv": np.ascontiguousarray(b[2 * D + c0: 2 * D + c0 + QKC]),
        })
    return in_maps


def assemble(results, B=B_FULL):
    out = np.empty((B, S, D), dtype=np.float32)
    for c in range(N_CORES):
        bi, hg = c // 2, c % 2
        out[bi, :, hg * QKC:(hg + 1) * QKC] = np.asarray(results[c]["outT"]).T
    return out


def kernel(inputs, mask, W_qkv, b_qkv):
    # mask is all-True for this problem (spec: fill=ones); it does not affect softmax.
    nc = _get_nc()
    in_maps = make_in_maps(inputs, W_qkv, b_qkv)
    res = run_bass_kernel_spmd(nc, in_maps, core_ids=list(range(N_CORES)))
    return assemble(res.results)


# revision 16
# speedup vs baseline: 1.2322x; 1.2322x over previous
"""Multi-head attention (B=4, S=2048, D=1024, H=16, HD=64) on 8 TRN2 NeuronCores.

Sharding: core c handles batch b = c//2 and head-group hg = c%2 (8 heads each).
Attention is embarrassingly parallel over (b, head-group); the QKV projection is
column-sharded per core (tensor parallel on heads).

Per-core dataflow (everything in "transposed" layout to avoid on-chip transposes):
  - Host passes X^T [D, S] (f32), W slices in natural [D, cols] layout.
  - Projection:  Q^T/K^T [1024, S] = W_qk^T @ X accumulated in SBUF tiles (sbt),
                 which the attention stage reads directly as Q^T/K^T;
                 V [S, 512] = X @ W_v, kept in SBUF augmented with a ones-column
                 per head (V').
  - Per head:    S^T[k,q] = K^T.T @ Q^T  (PSUM, fp32)
                 st = exp(S^T / 8)       (ScalarE, fused scale; mask is all-ones and
                                          softmax is shift-invariant => no max pass)
                 out^T[d,q], sums[q] = V'^T @ st  (ones-row of V' yields softmax sums)
                 out^T[d,q] /= sums[q]   (DVE reciprocal + gpsimd partition broadcast)
  - Host transposes per-core out^T [512, S] back and concatenates.

The projection is sliced into ~1us "quanta" (2 contraction chunks of one m-tile
half, or one V column-group of one sequence chunk) that are woven into the
attention chunk loop on a static schedule, so the in-order PE stream fills the
bubbles left by the ScalarE-paced softmax instead of running the projection as a
serial prefix. The schedule respects data deps: m-tile pair p is produced during
head 2p-1; V column-group g is produced just-in-time inside head 2g.

All matmuls run in float32r (fp32 data, ~1e-3 matmul rel err, bf16-class speed).
Projection partial sums accumulate in SBUF via DVE (float32r rounding per step).
b_qkv is applied (it is zeros in practice); mask is all-True per the problem spec
and is ignored.
"""

import numpy as np

import concourse.bass as bass
import concourse.mybir as mybir
import concourse.tile as tile
from concourse import bacc
from concourse.bass_utils import run_bass_kernel_spmd

F32 = mybir.dt.float32
F32R = mybir.dt.float32r
AF = mybir.ActivationFunctionType
ALU = mybir.AluOpType

P = 128          # partitions
D = 1024         # model dim
S = 2048         # sequence
HD = 64          # head dim
NHC = 8          # heads per core
QKC = NHC * HD   # 512 columns per core for each of Q, K, V
KD = D // P      # 8 contraction chunks
MS = S // P      # 16 sequence chunks
NQ = S // 512    # 4 q-tiles of 512
SCALE = 1.0 / 8.0  # 1/sqrt(HD)

N_CORES = 8
B_FULL, H_FULL = 4, 16


def _build(iters=1):
    nc = bacc.Bacc(None, target_bir_lowering=False)

    xt = nc.dram_tensor("xt", [D, S], F32R, kind="ExternalInput")
    wqk = nc.dram_tensor("wqk", [D, 2 * QKC], F32R, kind="ExternalInput")
    wv = nc.dram_tensor("wv", [D, QKC], F32R, kind="ExternalInput")
    bqk = nc.dram_tensor("bqk", [2 * QKC], F32, kind="ExternalInput")
    bv = nc.dram_tensor("bv", [QKC], F32, kind="ExternalInput")
    outT = nc.dram_tensor("outT", [QKC, S], F32, kind="ExternalOutput")

    with tile.TileContext(nc) as tc:
        with (
            tc.tile_pool(name="persist", bufs=1) as pp,
            tc.tile_pool(name="sbtp", bufs=4) as sbtp,
            tc.tile_pool(name="stp", bufs=5) as stp,
            tc.tile_pool(name="psc", bufs=2, space="PSUM") as psc,
            tc.tile_pool(name="psav", bufs=4, space="PSUM") as psav,
        ):
            # bias staging: bqk_sb[p, m] = bqk[m*128 + p]; bv broadcast across partitions
            bqk_sb = pp.tile([P, KD], F32, tag="bqk", name="bqk_sb")
            nc.sync.dma_start(out=bqk_sb[:], in_=bqk[:].rearrange("(m p) -> p m", p=P))
            bv_row = pp.tile([1, QKC], F32, tag="bvr", name="bv_row")
            nc.sync.dma_start(out=bv_row[:], in_=bv[:].rearrange("(o n) -> o n", o=1))
            bv_bc = pp.tile([P, QKC], F32, tag="bvb", name="bv_bc")
            nc.gpsimd.partition_broadcast(bv_bc[:], bv_row[:])
            ones8 = pp.tile([P, NHC], F32, tag="ones8", name="ones8")
            nc.vector.memset(ones8[:], 1.0)

            for it in range(iters):
                # V' tiles: [128 seq, 8 heads, 64+1] with ones in the last column
                v_sb = [
                    pp.tile([P, NHC, HD + 1], F32R, tag=f"v{k}", name=f"v{it}_{k}")
                    for k in range(MS)
                ]

                with tc.tile_pool(name=f"proj{it}", bufs=1) as pj:
                    w_tiles = {}

                    def load_wm(m, it=it):
                        w_tiles[m] = pj.tile([P, KD, P], F32R, tag="wm", bufs=2,
                                             name=f"wm{it}_{m}")
                        nc.sync.dma_start(
                            out=w_tiles[m][:],
                            in_=wqk[:, :].rearrange("(k p) n -> p k n", p=P)[:, :, m * P:(m + 1) * P])

                    load_wm(0)
                    load_wm(4)
                    xt_sb = [pj.tile([P, S], F32R, tag=f"xt{k}", name=f"xt{it}_{k}")
                             for k in range(KD)]
                    wv_sb = [pj.tile([P, QKC], F32R, tag=f"wv{k}", name=f"wv{it}_{k}")
                             for k in range(KD)]
                    for k in range(KD):
                        nc.sync.dma_start(out=xt_sb[k][:], in_=xt[k * P:(k + 1) * P, :])
                        nc.sync.dma_start(out=wv_sb[k][:], in_=wv[k * P:(k + 1) * P, :])

                    sbt_tiles = {}

                    def qk_quantum(m, nh, kk, it=it, k0=None, nk=4, first=None):
                        """Accumulate nk k-chunks (from k0) of m-tile m, n-half nh
                        into the sbt SBUF accumulator (PE -> PSUM -> DVE add)."""
                        if m not in w_tiles:
                            load_wm(m)
                        if m not in sbt_tiles:
                            sbt_tiles[m] = sbtp.tile([P, S], F32R, tag="sbt",
                                                     name=f"sbt{it}_{m}")
                        w_m, sbt = w_tiles[m], sbt_tiles[m]
                        if k0 is None:
                            k0 = 4 * kk
                        if first is None:
                            first = (kk == 0)
                        ps = psc.tile([P, 1024], F32, tag="sc", name=f"pq{it}_{m}_{nh}_{k0}")
                        for j, k in enumerate(range(k0, k0 + nk)):
                            nc.tensor.matmul(
                                ps[:, 0:512], w_m[:, k, :],
                                xt_sb[k][:, nh * 1024: nh * 1024 + 512],
                                start=(j == 0), stop=(j == nk - 1))
                            nc.tensor.matmul(
                                ps[:, 512:1024], w_m[:, k, :],
                                xt_sb[k][:, nh * 1024 + 512:(nh + 1) * 1024],
                                start=(j == 0), stop=(j == nk - 1))
                        dst = sbt[:, nh * 1024:(nh + 1) * 1024]
                        if first:
                            nc.vector.tensor_scalar_add(dst, ps[:], bqk_sb[:, m:m + 1])
                        else:
                            nc.vector.tensor_tensor(out=dst, in0=ps[:], in1=dst, op=ALU.add)

                    def v_quantum(ms, k0=0, nk=KD, first=True, it=it, v_sb=v_sb):
                        """Accumulate nk k-chunks of the V projection for sequence
                        chunk ms into the V' tile (all 8 heads, N=512)."""
                        ps = psc.tile([P, QKC], F32, tag="sc", name=f"pv{it}_{ms}_{k0}")
                        for j, k in enumerate(range(k0, k0 + nk)):
                            nc.tensor.matmul(
                                ps[:], xt_sb[k][:, ms * P:(ms + 1) * P], wv_sb[k][:],
                                start=(j == 0), stop=(j == nk - 1))
                        dst = v_sb[ms][:, :, 0:HD]
                        src3 = ps[:].rearrange("p (h e) -> p h e", e=HD)
                        if first:
                            nc.vector.tensor_tensor(
                                out=dst, in0=src3,
                                in1=bv_bc[:, :].rearrange("p (h e) -> p h e", e=HD),
                                op=ALU.add)
                            nc.vector.tensor_copy(
                                v_sb[ms][:, :, HD:HD + 1],
                                ones8[:, :].rearrange("p (h o) -> p h o", o=1))
                        else:
                            nc.vector.tensor_tensor(out=dst, in0=src3, in1=dst, op=ALU.add)

                    # ---- static quantum schedule ----
                    # sched[(h, kc)] -> quanta emitted inside that chunk, filling
                    # the PE bubble while ScalarE runs the chunk's exps. Only the
                    # remaining m-tile pairs are spread (thinly, ~0.4us/chunk) so
                    # the PE keeps slack to hide PSUM slot handoffs.
                    sched = {}

                    def add(h, kc, fn):
                        sched.setdefault((h, kc), []).append(fn)

                    # m-tile pair p (Q tile p, K tile 4+p) produced during heads
                    # 2p-2 / 2p-1, four quanta each at every 4th chunk
                    for p in (1, 2, 3):
                        quanta = []
                        for kk in range(2):
                            for m in (p, 4 + p):
                                for nh in range(2):
                                    quanta.append(lambda m=m, nh=nh, kk=kk: qk_quantum(m, nh, kk))
                        for i, fn in enumerate(quanta):
                            add(2 * p - 2 + i // 4, (4 * i) % 16, fn)

                    # prefix, ordered to trickle in behind the xt chunk loads:
                    # m0/m4 k-pairs and V k-halves start as their xt chunks land
                    for kk2 in range(2):
                        for m in (0, 4):
                            for nh in range(2):
                                qk_quantum(m, nh, None, k0=2 * kk2, nk=2, first=(kk2 == 0))
                    for ms in range(MS):
                        v_quantum(ms, k0=0, nk=4, first=True)
                    for kk2 in range(2, 4):
                        for m in (0, 4):
                            for nh in range(2):
                                qk_quantum(m, nh, None, k0=2 * kk2, nk=2, first=False)
                    for ms in range(MS):
                        v_quantum(ms, k0=4, nk=4, first=False)

                    # ---------------- attention ----------------
                    ot_cell = [None]

                    def attention_head(h, it=it, v_sb=v_sb):
                        g = h // 2
                        off = (h % 2) * HD
                        qt = sbt_tiles[g]
                        kt = sbt_tiles[4 + g]

                        avs = [
                            psav.tile([HD + 1, 512], F32, tag="av", name=f"av{it}_{h}_{q}")
                            for q in range(NQ)
                        ]
                        def emit_av(kc, st):
                            for q in range(NQ):
                                nc.tensor.matmul(
                                    avs[q][:], v_sb[kc][:, h, :], st[:, q * 512:(q + 1) * 512],
                                    start=(kc == 0), stop=(kc == MS - 1))

                        # software pipeline: chunk kc emits QK/exp for kc but the
                        # AV matmuls for kc-1, so the in-order PE stream never
                        # waits on ScalarE finishing the current chunk's exp.
                        prev = None
                        for kc in range(MS):
                            st = stp.tile([P, S], F32R, tag="st", name=f"st{it}_{h}_{kc}")
                            for qh in range(2):
                                sc = psc.tile([P, 1024], F32, tag="sc",
                                              name=f"sc{it}_{h}_{kc}_{qh}")
                                nc.tensor.matmul(
                                    sc[:, 0:512],
                                    kt[off:off + HD, kc * P:(kc + 1) * P],
                                    qt[off:off + HD, qh * 1024: qh * 1024 + 512],
                                    start=True, stop=True)
                                nc.tensor.matmul(
                                    sc[:, 512:1024],
                                    kt[off:off + HD, kc * P:(kc + 1) * P],
                                    qt[off:off + HD, qh * 1024 + 512:(qh + 1) * 1024],
                                    start=True, stop=True)
                                nc.scalar.activation(
                                    st[:, qh * 1024:(qh + 1) * 1024], sc[:],
                                    AF.Exp, scale=SCALE)
                            if prev is not None:
                                emit_av(*prev)
                            for fn in sched.pop((h, kc), ()):
                                fn()
                            prev = (kc, st)
                        emit_av(*prev)

                        # normalize: rows 0..63 of each av tile divided by the sums row
                        bc = stp.tile([HD, S], F32, tag="st", name=f"bc{it}_{h}")
                        for q in range(NQ):
                            rec = stp.tile([1, 512], F32, tag="rec", bufs=2,
                                           name=f"rec{it}_{h}_{q}")
                            nc.vector.reciprocal(rec[:], avs[q][HD:HD + 1, :])
                            nc.gpsimd.partition_broadcast(
                                bc[:, q * 512:(q + 1) * 512], rec[:])
                        if h % 2 == 0:
                            ot_cell[0] = stp.tile([P, S], F32, tag="st",
                                                  name=f"ot{it}_{h // 2}")
                        ot_g = ot_cell[0]
                        for q in range(NQ):
                            nc.vector.tensor_mul(
                                ot_g[off:off + HD, q * 512:(q + 1) * 512],
                                avs[q][0:HD, :], bc[:, q * 512:(q + 1) * 512])
                        if h % 2 == 1:
                            gg = h // 2
                            nc.sync.dma_start(out=outT[gg * P:(gg + 1) * P, :], in_=ot_g[:])

                    for h in range(NHC):
                        attention_head(h)
                    assert not sched, f"unemitted quanta: {list(sched)}"

    nc.finalize()
    return nc


_NC_CACHE = {}


def _get_nc(iters=1):
    if iters not in _NC_CACHE:
        _NC_CACHE[iters] = _build(iters)
    return _NC_CACHE[iters]


def make_in_maps(inputs, W_qkv, b_qkv):
    inputs = np.asarray(inputs, dtype=np.float32)
    W = np.asarray(W_qkv, dtype=np.float32)
    b = np.asarray(b_qkv, dtype=np.float32)
    xt_by_b = [np.ascontiguousarray(inputs[bi].T) for bi in range(B_FULL)]
    in_maps = []
    for c in range(N_CORES):
        bi, hg = c // 2, c % 2
        c0 = hg * QKC
        in_maps.append({
            "xt": xt_by_b[bi],
            "wqk": np.ascontiguousarray(
                np.concatenate([W[:, c0:c0 + QKC], W[:, D + c0: D + c0 + QKC]], axis=1)),
            "wv": np.ascontiguousarray(W[:, 2 * D + c0: 2 * D + c0 + QKC]),
            "bqk": np.ascontiguousarray(
                np.concatenate([b[c0:c0 + QKC], b[D + c0: D + c0 + QKC]])),
            "bv": np.ascontiguousarray(b[2 * D + c0: 2 * D + c0 + QKC]),
        })
    return in_maps


def assemble(results, B=B_FULL):
    out = np.empty((B, S, D), dtype=np.float32)
    for c in range(N_CORES):
        bi, hg = c // 2, c % 2
        out[bi, :, hg * QKC:(hg + 1) * QKC] = np.asarray(results[c]["outT"]).T
    return out


def kernel(inputs, mask, W_qkv, b_qkv):
    # mask is all-True for this problem (spec: fill=ones); it does not affect softmax.
    nc = _get_nc()
    in_maps = make_in_maps(inputs, W_qkv, b_qkv)
    res = run_bass_kernel_spmd(nc, in_maps, core_ids=list(range(N_CORES)))
    return assemble(res.results)


# revision 18
# speedup vs baseline: 278.3766x; 225.9198x over previous
"""Multi-head attention (B=4, S=2048, D=1024, H=16, HD=64) on 8 TRN2 NeuronCores.

Sharding: core c handles batch b = c//2 and head-group hg = c%2 (8 heads each).
Attention is embarrassingly parallel over (b, head-group); the QKV projection is
column-sharded per core (tensor parallel on heads).

Per-core dataflow (everything in "transposed" layout to avoid on-chip transposes):
  - Host passes X^T [D, S] (f32), W slices in natural [D, cols] layout.
  - Projection:  Q^T/K^T [1024, S] = W_qk^T @ X accumulated in SBUF tiles (sbt),
                 which the attention stage reads directly as Q^T/K^T;
                 V [S, 512] = X @ W_v, kept in SBUF augmented with a ones-column
                 per head (V').
  - Per head:    S^T[k,q] = K^T.T @ Q^T  (PSUM, fp32)
                 st = exp(S^T / 8)       (ScalarE, fused scale; mask is all-ones and
                                          softmax is shift-invariant => no max pass)
                 out^T[d,q], sums[q] = V'^T @ st  (ones-row of V' yields softmax sums)
                 out^T[d,q] /= sums[q]   (DVE reciprocal + gpsimd partition broadcast)
  - Host transposes per-core out^T [512, S] back and concatenates.

The projection is sliced into ~1us "quanta" (2 contraction chunks of one m-tile
half, or one V column-group of one sequence chunk) that are woven into the
attention chunk loop on a static schedule, so the in-order PE stream fills the
bubbles left by the ScalarE-paced softmax instead of running the projection as a
serial prefix. The schedule respects data deps: m-tile pair p is produced during
head 2p-1; V column-group g is produced just-in-time inside head 2g.

All matmuls run in float32r (fp32 data, ~1e-3 matmul rel err, bf16-class speed).
Projection partial sums accumulate in SBUF via DVE (float32r rounding per step).
b_qkv is applied (it is zeros in practice); mask is all-True per the problem spec
and is ignored.
"""

import numpy as np

import concourse.bass as bass
import concourse.mybir as mybir
import concourse.tile as tile
from concourse import bacc
from concourse.bass_utils import run_bass_kernel_spmd

F32 = mybir.dt.float32
F32R = mybir.dt.float32r
AF = mybir.ActivationFunctionType
ALU = mybir.AluOpType

P = 128          # partitions
D = 1024         # model dim
S = 2048         # sequence
HD = 64          # head dim
NHC = 8          # heads per core
QKC = NHC * HD   # 512 columns per core for each of Q, K, V
KD = D // P      # 8 contraction chunks
MS = S // P      # 16 sequence chunks
NQ = S // 512    # 4 q-tiles of 512
SCALE = 1.0 / 8.0  # 1/sqrt(HD)

N_CORES = 8
B_FULL, H_FULL = 4, 16


def _build(iters=1):
    nc = bacc.Bacc(None, target_bir_lowering=False)

    xt = nc.dram_tensor("xt", [D, S], F32R, kind="ExternalInput")
    # wqk is host-permuted: row (m*128 + p), col (k*128 + j) holds
    # W_qk[k*128 + p, m*128 + j] — so one m-tile's weights are a contiguous
    # [128, 1024] block (4KB DMA lines instead of 512B strided reads)
    wqk = nc.dram_tensor("wqk", [D, 2 * QKC], F32R, kind="ExternalInput")
    wv = nc.dram_tensor("wv", [D, QKC], F32R, kind="ExternalInput")
    bqk = nc.dram_tensor("bqk", [2 * QKC], F32, kind="ExternalInput")
    bv = nc.dram_tensor("bv", [QKC], F32, kind="ExternalInput")
    outT = nc.dram_tensor("outT", [QKC, S], F32, kind="ExternalOutput")

    with tile.TileContext(nc) as tc:
        with (
            tc.tile_pool(name="persist", bufs=1) as pp,
            tc.tile_pool(name="sbtp", bufs=4) as sbtp,
            tc.tile_pool(name="stp", bufs=5) as stp,
            tc.tile_pool(name="psc", bufs=2, space="PSUM") as psc,
            tc.tile_pool(name="psav", bufs=4, space="PSUM") as psav,
        ):
            # bias staging: bqk_sb[p, m] = bqk[m*128 + p]; bv broadcast across partitions
            bqk_sb = pp.tile([P, KD], F32, tag="bqk", name="bqk_sb")
            nc.sync.dma_start(out=bqk_sb[:], in_=bqk[:].rearrange("(m p) -> p m", p=P))
            bv_row = pp.tile([1, QKC], F32, tag="bvr", name="bv_row")
            nc.sync.dma_start(out=bv_row[:], in_=bv[:].rearrange("(o n) -> o n", o=1))
            bv_bc = pp.tile([P, QKC], F32, tag="bvb", name="bv_bc")
            nc.gpsimd.partition_broadcast(bv_bc[:], bv_row[:])
            ones8 = pp.tile([P, NHC], F32, tag="ones8", name="ones8")
            nc.vector.memset(ones8[:], 1.0)

            for it in range(iters):
                # V' tiles: [128 seq, 8 heads, 64+1] with ones in the last column
                v_sb = [
                    pp.tile([P, NHC, HD + 1], F32R, tag=f"v{k}", name=f"v{it}_{k}")
                    for k in range(MS)
                ]

                with tc.tile_pool(name=f"proj{it}", bufs=1) as pj:
                    w_tiles = {}

                    def load_wm(m, it=it):
                        w_tiles[m] = pj.tile([P, KD, P], F32R, tag="wm", bufs=2,
                                             name=f"wm{it}_{m}")
                        nc.sync.dma_start(
                            out=w_tiles[m][:],
                            in_=wqk[m * P:(m + 1) * P, :].rearrange("p (k j) -> p k j", k=KD))

                    load_wm(0)
                    load_wm(4)
                    xt_sb = [pj.tile([P, S], F32R, tag=f"xt{k}", name=f"xt{it}_{k}")
                             for k in range(KD)]
                    wv_sb = [pj.tile([P, QKC], F32R, tag=f"wv{k}", name=f"wv{it}_{k}")
                             for k in range(KD)]
                    # order: first 4 xt chunks (unblocks the first m0/m4 quanta),
                    # then wv 0-3 (unblocks V k-half 0), then the rest
                    for k in range(4):
                        nc.sync.dma_start(out=xt_sb[k][:], in_=xt[k * P:(k + 1) * P, :])
                    for k in range(4):
                        nc.sync.dma_start(out=wv_sb[k][:], in_=wv[k * P:(k + 1) * P, :])
                    for k in range(4, KD):
                        nc.sync.dma_start(out=xt_sb[k][:], in_=xt[k * P:(k + 1) * P, :])
                    for k in range(4, KD):
                        nc.sync.dma_start(out=wv_sb[k][:], in_=wv[k * P:(k + 1) * P, :])

                    sbt_tiles = {}

                    def qk_quantum(m, nh, kk, it=it, k0=None, nk=4, first=None):
                        """Accumulate nk k-chunks (from k0) of m-tile m, n-half nh
                        into the sbt SBUF accumulator (PE -> PSUM -> DVE add)."""
                        if m not in w_tiles:
                            load_wm(m)
                        if m not in sbt_tiles:
                            sbt_tiles[m] = sbtp.tile([P, S], F32R, tag="sbt",
                                                     name=f"sbt{it}_{m}")
                        w_m, sbt = w_tiles[m], sbt_tiles[m]
                        if k0 is None:
                            k0 = 4 * kk
                        if first is None:
                            first = (kk == 0)
                        ps = psc.tile([P, 1024], F32, tag="sc", name=f"pq{it}_{m}_{nh}_{k0}")
                        for j, k in enumerate(range(k0, k0 + nk)):
                            nc.tensor.matmul(
                                ps[:, 0:512], w_m[:, k, :],
                                xt_sb[k][:, nh * 1024: nh * 1024 + 512],
                                start=(j == 0), stop=(j == nk - 1))
                            nc.tensor.matmul(
                                ps[:, 512:1024], w_m[:, k, :],
                                xt_sb[k][:, nh * 1024 + 512:(nh + 1) * 1024],
                                start=(j == 0), stop=(j == nk - 1))
                        dst = sbt[:, nh * 1024:(nh + 1) * 1024]
                        if first:
                            nc.vector.tensor_scalar_add(dst, ps[:], bqk_sb[:, m:m + 1])
                        else:
                            nc.vector.tensor_tensor(out=dst, in0=ps[:], in1=dst, op=ALU.add)

                    def v_quantum(ms, k0=0, nk=KD, first=True, it=it, v_sb=v_sb):
                        """Accumulate nk k-chunks of the V projection for sequence
                        chunk ms into the V' tile (all 8 heads, N=512)."""
                        ps = psc.tile([P, QKC], F32, tag="sc", name=f"pv{it}_{ms}_{k0}")
                        for j, k in enumerate(range(k0, k0 + nk)):
                            nc.tensor.matmul(
                                ps[:], xt_sb[k][:, ms * P:(ms + 1) * P], wv_sb[k][:],
                                start=(j == 0), stop=(j == nk - 1))
                        dst = v_sb[ms][:, :, 0:HD]
                        src3 = ps[:].rearrange("p (h e) -> p h e", e=HD)
                        if first:
                            nc.vector.tensor_tensor(
                                out=dst, in0=src3,
                                in1=bv_bc[:, :].rearrange("p (h e) -> p h e", e=HD),
                                op=ALU.add)
                            nc.vector.tensor_copy(
                                v_sb[ms][:, :, HD:HD + 1],
                                ones8[:, :].rearrange("p (h o) -> p h o", o=1))
                        else:
                            nc.vector.tensor_tensor(out=dst, in0=src3, in1=dst, op=ALU.add)

                    # ---- static quantum schedule ----
                    # sched[(h, kc)] -> quanta emitted inside that chunk, filling
                    # the PE bubble while ScalarE runs the chunk's exps. Only the
                    # remaining m-tile pairs are spread (thinly, ~0.4us/chunk) so
                    # the PE keeps slack to hide PSUM slot handoffs.
                    sched = {}

                    def add(h, kc, fn):
                        sched.setdefault((h, kc), []).append(fn)

                    # m-tile pair p (Q tile p, K tile 4+p) produced during heads
                    # 2p-2 / 2p-1, four quanta each at every 4th chunk
                    for p in (1, 2, 3):
                        quanta = []
                        for kk in range(2):
                            for m in (p, 4 + p):
                                for nh in range(2):
                                    quanta.append(lambda m=m, nh=nh, kk=kk: qk_quantum(m, nh, kk))
                        for i, fn in enumerate(quanta):
                            add(2 * p - 2 + i // 4, (4 * i) % 16, fn)

                    # prefix, ordered to trickle in behind the xt chunk loads:
                    # m0/m4 k-pairs and V k-halves start as their xt chunks land
                    for kk2 in range(2):
                        for m in (0, 4):
                            for nh in range(2):
                                qk_quantum(m, nh, None, k0=2 * kk2, nk=2, first=(kk2 == 0))
                    for ms in range(MS):
                        v_quantum(ms, k0=0, nk=4, first=True)
                    for kk2 in range(2, 4):
                        for m in (0, 4):
                            for nh in range(2):
                                qk_quantum(m, nh, None, k0=2 * kk2, nk=2, first=False)
                    for ms in range(MS):
                        v_quantum(ms, k0=4, nk=4, first=False)

                    # ---------------- attention ----------------
                    ot_cell = [None]

                    def attention_head(h, it=it, v_sb=v_sb):
                        g = h // 2
                        off = (h % 2) * HD
                        qt = sbt_tiles[g]
                        kt = sbt_tiles[4 + g]

                        avs = [
                            psav.tile([HD + 1, 512], F32, tag="av", name=f"av{it}_{h}_{q}")
                            for q in range(NQ)
                        ]
                        def emit_av(kc, st):
                            for q in range(NQ):
                                nc.tensor.matmul(
                                    avs[q][:], v_sb[kc][:, h, :], st[:, q * 512:(q + 1) * 512],
                                    start=(kc == 0), stop=(kc == MS - 1))

                        # software pipeline: chunk kc emits QK/exp for kc but the
                        # AV matmuls for kc-1, so the in-order PE stream never
                        # waits on ScalarE finishing the current chunk's exp.
                        prev = None
                        for kc in range(MS):
                            st = stp.tile([P, S], F32R, tag="st", name=f"st{it}_{h}_{kc}")
                            for qh in range(2):
                                sc = psc.tile([P, 1024], F32, tag="sc",
                                              name=f"sc{it}_{h}_{kc}_{qh}")
                                nc.tensor.matmul(
                                    sc[:, 0:512],
                                    kt[off:off + HD, kc * P:(kc + 1) * P],
                                    qt[off:off + HD, qh * 1024: qh * 1024 + 512],
                                    start=True, stop=True)
                                nc.tensor.matmul(
                                    sc[:, 512:1024],
                                    kt[off:off + HD, kc * P:(kc + 1) * P],
                                    qt[off:off + HD, qh * 1024 + 512:(qh + 1) * 1024],
                                    start=True, stop=True)
                                nc.scalar.activation(
                                    st[:, qh * 1024:(qh + 1) * 1024], sc[:],
                                    AF.Exp, scale=SCALE)
                            if prev is not None:
                                emit_av(*prev)
                            for fn in sched.pop((h, kc), ()):
                                fn()
                            prev = (kc, st)
                        emit_av(*prev)

                        # normalize: rows 0..63 of each av tile divided by the sums row
                        bc = stp.tile([HD, S], F32, tag="st", name=f"bc{it}_{h}")
                        for q in range(NQ):
                            rec = stp.tile([1, 512], F32, tag="rec", bufs=2,
                                           name=f"rec{it}_{h}_{q}")
                            nc.vector.reciprocal(rec[:], avs[q][HD:HD + 1, :])
                            nc.gpsimd.partition_broadcast(
                                bc[:, q * 512:(q + 1) * 512], rec[:])
                        if h % 2 == 0:
                            ot_cell[0] = stp.tile([P, S], F32, tag="st",
                                                  name=f"ot{it}_{h // 2}")
                        ot_g = ot_cell[0]
                        for q in range(NQ):
                            nc.vector.tensor_mul(
                                ot_g[off:off + HD, q * 512:(q + 1) * 512],
                                avs[q][0:HD, :], bc[:, q * 512:(q + 1) * 512])
                        if h % 2 == 1:
                            gg = h // 2
                            nc.sync.dma_start(out=outT[gg * P:(gg + 1) * P, :], in_=ot_g[:])

                    for h in range(NHC):
                        attention_head(h)
                    assert not sched, f"unemitted quanta: {list(sched)}"

    nc.finalize()
    return nc


_NC_CACHE = {}


def _get_nc(iters=1):
    if iters not in _NC_CACHE:
        _NC_CACHE[iters] = _build(iters)
    return _NC_CACHE[iters]


def _permute_wqk(wqk):
    # [k*128+p, m*128+j] -> [m*128+p, k*128+j]: one m-tile contiguous per row
    w4 = wqk.reshape(KD, P, KD, P)
    return np.ascontiguousarray(w4.transpose(2, 1, 0, 3).reshape(D, D))


def make_in_maps(inputs, W_qkv, b_qkv):
    inputs = np.asarray(inputs, dtype=np.float32)
    W = np.asarray(W_qkv, dtype=np.float32)
    b = np.asarray(b_qkv, dtype=np.float32)
    xt_by_b = [np.ascontiguousarray(inputs[bi].T) for bi in range(B_FULL)]
    in_maps = []
    for c in range(N_CORES):
        bi, hg = c // 2, c % 2
        c0 = hg * QKC
        in_maps.append({
            "xt": xt_by_b[bi],
            "wqk": _permute_wqk(
                np.concatenate([W[:, c0:c0 + QKC], W[:, D + c0: D + c0 + QKC]], axis=1)),
            "wv": np.ascontiguousarray(W[:, 2 * D + c0: 2 * D + c0 + QKC]),
            "bqk": np.ascontiguousarray(
                np.concatenate([b[c0:c0 + QKC], b[D + c0: D + c0 + QKC]])),
            "bv": np.ascontiguousarray(b[2 * D + c0: 2 * D + c0 + QKC]),
        })
    return in_maps


def assemble(results, B=B_FULL):
    out = np.empty((B, S, D), dtype=np.float32)
    for c in range(N_CORES):
        bi, hg = c // 2, c % 2
        out[bi, :, hg * QKC:(hg + 1) * QKC] = np.asarray(results[c]["outT"]).T
    return out


def kernel(inputs, mask, W_qkv, b_qkv):
    # mask is all-True for this problem (spec: fill=ones); it does not affect softmax.
    nc = _get_nc()
    in_maps = make_in_maps(inputs, W_qkv, b_qkv)
    res = run_bass_kernel_spmd(nc, in_maps, core_ids=list(range(N_CORES)))
    return assemble(res.results)
